# revision 27
# baseline (speedup 1.0000x reference)
"""Trainium2 Bass kernel for nn_NonSpikingOutput.

Reference semantics (N=4096 neurons, O=3 outputs, T=4096 steps):
    g_max = k/(e-k); act = clip(u, 0, 1); i_syn = g_max*act*(e - v)
    RK2 with i_syn frozen collapses to the per-element linear recurrence
        v_t = A_t * v_{t-1} + 0.075 * c_t * e_t,   A_t = 0.625 - 0.075*c_t
        c_t = act_t * k_t / (e_t - k_t)
    out[o, t] = sum_n v[n, o, t]

v4 design (measured HW rates per (128,2048) tile: TT bf16 1.22us, TT
f32/mixed 2.28us, TS bf16 0.68us, scalar_tensor_tensor/custom-DVE 2.28us
always, scan 4.39us any dtype, ACT 2.0us any dtype):

  DVE:  GMAX custom op  g~ = -0.075*k*recip_1nr(e-k)   [1 op, 2.28us]
        c~ = g~*act (bf16 TT), B~ = c~*eb (bf16 TT), scan(A, B~),
        act = clip(u) via dual TS (shared over the 3 outputs)
  ACT:  eb = bf16(e), A = c~ + 0.625 (Copy w/ bias), psum evac, carries
  PE:   colsum via (-1)-weights matmul -> psum = +0.075*sum_n v
Scans are software-pipelined one unit behind the TT chain so the ACT
A-compute latency never stalls the DVE.

Work scaled by -0.075: w~ = A*w~ + B~ accumulates -0.075*v, and the -1
matmul weights flip the sign back while summing over neurons.

Sharding: neuron dim N split across 8 cores (512 each); host sums the
per-core (O, T) partials.
"""

import sys
from contextlib import ExitStack

import numpy as np

sys.path.insert(0, "/opt/trn_rl_repo")

import concourse.bass as bass
import concourse.tile as tile
from concourse import bacc, mybir
from concourse.bass_utils import run_bass_kernel_spmd
from concourse import dve_ops as dops
from concourse.dve_spec import Spec, Bin, AluOp, Src0, Src1, C0, C1, C2, lower
from concourse.dve_spec import _has_src1 as _has_src1
from concourse.dve_uop import DveOpSpec

N_CORES = 8
N, O, T = 4096, 3, 4096
NL = N // N_CORES  # neurons per core
NG = NL // 128     # 128-partition neuron groups per core
F = 2048           # time-chunk (free dim) per tile
TC = T // F
FP32 = mybir.dt.float32
BF16 = mybir.dt.bfloat16
OP = mybir.AluOpType
AF = mybir.ActivationFunctionType


def _register_op(name: str, spec: Spec, subdim: bool = False):
    """Runtime-register a custom DVE op (name -> free opcode row)."""
    for o in dops.OPS:
        if o.name == name:
            return o
    row = dops._CUSTOM_DVE_ROW_BASE + len(dops.OPS)
    assert row < 0x20, "out of custom-DVE opcode rows"
    shas = {}
    for ver in ("v3", "v4"):
        try:
            uops = lower(spec, ver=ver)
            shas[ver] = DveOpSpec(
                name=name, opcode=row, uops=uops, rd1_en=_has_src1(spec)
            ).sha(ver)
        except Exception:
            pass
    assert "v3" in shas, "v3 lowering must succeed for TRN2"
    op = dops.DveOp(name, spec, subdim, shas)
    dops.OPS.append(op)
    dops.CUSTOM_DVE_SPECS[name] = spec
    dops._SUB_OPCODE_FOR_NAME[name] = row
    return op


# g~ = C2 * k * recip(e - k); recip = BITWISE_NOT exponent-flip seed + one
# Newton step (max rel err 1.7e-3 on d in (1,3); C0/C1 are the minimax pair).
_x = Src1 - Src0
_nx = Bin(AluOp.BITWISE_NOT, _x, _x)
_y0 = _nx * C0
_y1 = _y0 * (C1 - _x * _y0)
_GMAX_BODY = (Src0 * _y1) * C2


def _ref_gmax(in0, in1, c0, c1, c2):
    x = (in1 - in0).astype(np.float32)
    nx = (~x.view(np.int32)).view(np.float32)
    y0 = nx * np.float32(c0)
    y1 = y0 * (np.float32(c1) - x * y0)
    return (in0 * y1) * np.float32(c2)


GMAX = _register_op("GMAX_RECIP_ANT", Spec(body=_GMAX_BODY, reference=_ref_gmax))
GMAX_C = {"s0": -0.23549792, "s1": 2.0017324, "imm2": -0.075}


def _build_nc_v4() -> bass.Bass:
    nc = bacc.Bacc(
        "TRN2", target_bir_lowering=False, debug=False, num_devices=N_CORES
    )
    u = nc.dram_tensor("u", [NL, T], FP32, kind="ExternalInput")
    k = nc.dram_tensor("k", [NL, O, T], FP32, kind="ExternalInput")
    e = nc.dram_tensor("e", [NL, O, T], FP32, kind="ExternalInput")
    out = nc.dram_tensor("out", [O, T], FP32, kind="ExternalOutput")

    with tile.TileContext(nc) as tc, ExitStack() as ctx:
        const_pool = ctx.enter_context(tc.tile_pool(name="const", bufs=1))
        negones = const_pool.tile([128, 1], BF16)
        nc.vector.memset(negones[:], -1.0)
        # one fp32 carry column per (o, g) + one scratch col for strip chains
        carry = const_pool.tile([128, O * NG + 1], FP32)
        SCR = O * NG

        u_pool = ctx.enter_context(tc.tile_pool(name="u", bufs=1))
        act_pool = ctx.enter_context(tc.tile_pool(name="act", bufs=2 * NG))
        k_pool = ctx.enter_context(tc.tile_pool(name="k", bufs=3))
        e_pool = ctx.enter_context(tc.tile_pool(name="e", bufs=3))
        eb_pool = ctx.enter_context(tc.tile_pool(name="eb", bufs=3))
        g_pool = ctx.enter_context(tc.tile_pool(name="g", bufs=3))
        c_pool = ctx.enter_context(tc.tile_pool(name="c", bufs=3))
        a_pool = ctx.enter_context(tc.tile_pool(name="a", bufs=4))
        b_pool = ctx.enter_context(tc.tile_pool(name="b", bufs=4))
        w_pool = ctx.enter_context(tc.tile_pool(name="w", bufs=4))
        r_pool = ctx.enter_context(tc.tile_pool(name="r", bufs=1))
        ps_pool = ctx.enter_context(tc.tile_pool(name="ps", bufs=2, space="PSUM"))

        acts: dict[int, object] = {}
        acts_next: dict[int, object] = {}
        # software pipeline: scans run DEPTH units behind the TT chain so
        # every scan's inputs (incl. the ACT-produced A) are long-posted
        # by the time the scan issues - avoids per-scan semaphore latency.
        DEPTH = 2
        pending = []  # (A, B, w, ci, tci, o, g, ps)

        evacs = []  # deferred psum evacuations (deadline: ps reuse, far away)

        def drain_evac():
            pps, po, pt0 = evacs.pop(0)
            row = r_pool.tile([1, F], FP32, tag="row")
            nc.scalar.copy(row[:], pps[:])
            nc.sync.dma_start(out[po : po + 1, pt0 : pt0 + F], row[:, :])

        def flush_one():
            At, Bt, wt, ci, tci, o, g, ps = pending.pop(0)
            init = 0.0 if tci == 0 else carry[:, ci : ci + 1]
            nc.vector.tensor_tensor_scan(wt[:], At[:], Bt[:], init, OP.mult, OP.add)
            if tci < TC - 1:
                # carry on DVE ([128,1] ~0.1us): an ACT-side copy would wait
                # on the delayed scan INSIDE the ACT FIFO, head-of-line
                # blocking every later eb/A copy behind it
                nc.vector.tensor_scalar(
                    carry[:, ci : ci + 1], wt[:, F - 1 : F], 1.0, None, OP.mult
                )
            for s in range(F // 512):
                nc.tensor.matmul(
                    ps[0:1, s * 512 : (s + 1) * 512],
                    negones[:],
                    wt[:, s * 512 : (s + 1) * 512],
                    start=(g == 0),
                    stop=(g == NG - 1),
                )
            if g == NG - 1:
                if tci == TC - 1 and o == O - 1:
                    # final group: evacuate per-512 chunk right behind each
                    # matmul so the tail chain pipelines instead of
                    # serializing scan->4xmm->evac->dma
                    row = r_pool.tile([1, F], FP32, tag="row")
                    for s in range(F // 512):
                        sl = slice(s * 512, (s + 1) * 512)
                        nc.scalar.copy(row[:, sl], ps[0:1, sl])
                        nc.sync.dma_start(
                            out[o : o + 1, tci * F + s * 512 : tci * F + (s + 1) * 512],
                            row[:, sl],
                        )
                else:
                    evacs.append((ps, o, tci * F))
                    if len(evacs) > 1:
                        drain_evac()

        for tci in range(TC):
            t0 = tci * F
            if tci > 0:
                acts = dict(acts_next)
                acts_next = {}
            for o in range(O):
                ps = ps_pool.tile([1, F], FP32, tag="ps", name=f"ps{tci}_{o}")
                for g in range(NG):
                    p0 = g * 128
                    kt = k_pool.tile([128, F], FP32, tag="k")
                    nc.sync.dma_start(kt[:], k[p0 : p0 + 128, o, t0 : t0 + F])
                    # e rides the Activation engine's HWDGE queue: splitting
                    # the 59MB input stream across two hardware queues keeps
                    # both k and e ahead of compute (one queue saturates)
                    et = e_pool.tile([128, F], FP32, tag="e")
                    nc.scalar.dma_start(et[:], e[p0 : p0 + 128, o, t0 : t0 + F])
                    if o == 0 and tci == 0:
                        ut = u_pool.tile([128, F], FP32, tag="u")
                        nc.sync.dma_start(ut[:], u[p0 : p0 + 128, t0 : t0 + F])

                    eb = eb_pool.tile([128, F], BF16, tag="eb")
                    nc.scalar.copy(eb[:], et[:])

                    gt = g_pool.tile([128, F], BF16, tag="g")
                    nc.vector._custom_dve(
                        GMAX, out=gt[:], in0=kt[:], in1=et[:], **GMAX_C
                    )
                    if o == 0 and tci == 0:
                        # u -> bf16 on ACT so the clip TS runs in 4x mode
                        ub = eb_pool.tile([128, F], BF16, tag="ub", name=f"ub{tci}_{g}")
                        nc.scalar.copy(ub[:], ut[:])
                        av = act_pool.tile([128, F], BF16, tag="act")
                        nc.vector.tensor_scalar(av[:], ub[:], 0.0, 1.0, OP.max, OP.min)
                        acts[g] = av
                    ct = c_pool.tile([128, F], BF16, tag="c")
                    nc.vector.tensor_tensor(ct[:], gt[:], acts[g][:], OP.mult)
                    # A = c~ + 0.625 on ACT (Copy w/ bias) to keep DVE lean
                    At = a_pool.tile([128, F], BF16, tag="a")
                    nc.scalar.activation(At[:], ct[:], AF.Copy, bias=0.625, scale=1.0)
                    Bt = b_pool.tile([128, F], BF16, tag="b")
                    nc.vector.tensor_tensor(Bt[:], ct[:], eb[:], OP.mult)

                    wt = w_pool.tile([128, F], BF16, tag="w")
                    ci = o * NG + g
                    pending.append((At, Bt, wt, ci, tci, o, g, ps))
                    # steady-state: flush DEPTH units behind; in the final
                    # group drain eagerly so trailing scans overlap the
                    # remaining TT work instead of serializing at the end.
                    last_group = tci == TC - 1 and o == O - 1
                    while len(pending) > (0 if last_group else DEPTH):
                        flush_one()
                    # penultimate psum evac off the tail: by mid last group
                    # its matmuls are done, ACT is idling
                    if last_group and g == 2 and evacs:
                        drain_evac()
                    # prefetch next chunk's u + clip during this chunk's
                    # final output group: spreads the ACT/DVE o==0 burst
                    if o == O - 1 and tci < TC - 1:
                        nt0 = (tci + 1) * F
                        ut = u_pool.tile([128, F], FP32, tag="u")
                        nc.sync.dma_start(ut[:], u[p0 : p0 + 128, nt0 : nt0 + F])
                        ub = eb_pool.tile(
                            [128, F], BF16, tag="ub", name=f"ub{tci + 1}_{g}"
                        )
                        nc.scalar.copy(ub[:], ut[:])
                        av = act_pool.tile([128, F], BF16, tag="act")
                        nc.vector.tensor_scalar(av[:], ub[:], 0.0, 1.0, OP.max, OP.min)
                        acts_next[g] = av
        while pending:
            flush_one()
        while evacs:
            drain_evac()

    nc.compile()
    return nc


def _build_nc_v13() -> bass.Bass:
    """Full-T scans: one scan of (128, 4096) per (o, g) - no carry chain,
    half the scan launches, no chunk boundary. k/e/eb/g/c stay at F=2048
    (two halves per unit); A/B/w are double-width tiles whose halves are
    written independently, then consumed whole by the scan."""
    nc = bacc.Bacc(
        "TRN2", target_bir_lowering=False, debug=False, num_devices=N_CORES
    )
    u = nc.dram_tensor("u", [NL, T], FP32, kind="ExternalInput")
    k = nc.dram_tensor("k", [NL, O, T], FP32, kind="ExternalInput")
    e = nc.dram_tensor("e", [NL, O, T], FP32, kind="ExternalInput")
    out = nc.dram_tensor("out", [O, T], FP32, kind="ExternalOutput")

    with tile.TileContext(nc) as tc, ExitStack() as ctx:
        const_pool = ctx.enter_context(tc.tile_pool(name="const", bufs=1))
        negones = const_pool.tile([128, 1], BF16)
        nc.vector.memset(negones[:], -1.0)

        u_pool = ctx.enter_context(tc.tile_pool(name="u", bufs=1))
        ub_pool = ctx.enter_context(tc.tile_pool(name="ub", bufs=1))
        act_pool = ctx.enter_context(tc.tile_pool(name="act", bufs=NG))
        k_pool = ctx.enter_context(tc.tile_pool(name="k", bufs=3))
        e_pool = ctx.enter_context(tc.tile_pool(name="e", bufs=3))
        eb_pool = ctx.enter_context(tc.tile_pool(name="eb", bufs=3))
        g_pool = ctx.enter_context(tc.tile_pool(name="g", bufs=3))
        c_pool = ctx.enter_context(tc.tile_pool(name="c", bufs=3))
        a_pool = ctx.enter_context(tc.tile_pool(name="a", bufs=2))
        b_pool = ctx.enter_context(tc.tile_pool(name="b", bufs=2))
        w_pool = ctx.enter_context(tc.tile_pool(name="w", bufs=2))
        r_pool = ctx.enter_context(tc.tile_pool(name="r", bufs=1))
        ps_pool = ctx.enter_context(tc.tile_pool(name="ps", bufs=2, space="PSUM"))

        acts: dict[int, object] = {}
        pending = []  # (Abig, Bbig, wbig, g, ps)

        def flush_one():
            Abig, Bbig, wbig, oo, g, ps = pending.pop(0)
            nc.vector.tensor_tensor_scan(
                wbig[:], Abig[:], Bbig[:], 0.0, OP.mult, OP.add
            )
            for s in range(T // 512):
                nc.tensor.matmul(
                    ps[s // 4][0:1, (s % 4) * 512 : (s % 4) * 512 + 512],
                    negones[:],
                    wbig[:, s * 512 : (s + 1) * 512],
                    start=(g == 0),
                    stop=(g == NG - 1),
                )
            if g == NG - 1:
                for h in range(2):
                    row = r_pool.tile([1, F], FP32, tag="row")
                    nc.scalar.copy(row[:], ps[h][:])
                    nc.sync.dma_start(
                        out[oo : oo + 1, h * F : h * F + F], row[:, :]
                    )

        for o in range(O):
            ps = [
                ps_pool.tile([1, F], FP32, tag="ps", name=f"ps{o}_{h}")
                for h in range(2)
            ]
            for g in range(NG):
                p0 = g * 128
                if o == 0:
                    ut = u_pool.tile([128, T], FP32, tag="u")
                    nc.sync.dma_start(ut[:], u[p0 : p0 + 128, :])
                    ub = ub_pool.tile([128, T], BF16, tag="ub", name=f"ub{g}")
                    nc.scalar.copy(ub[:], ut[:])
                    av = act_pool.tile([128, T], BF16, tag="act")
                    nc.vector.tensor_scalar(av[:], ub[:], 0.0, 1.0, OP.max, OP.min)
                    acts[g] = av
                Abig = a_pool.tile([128, T], BF16, tag="a")
                Bbig = b_pool.tile([128, T], BF16, tag="b")
                for h in range(2):
                    t0 = h * F
                    kt = k_pool.tile([128, F], FP32, tag="k")
                    nc.sync.dma_start(kt[:], k[p0 : p0 + 128, o, t0 : t0 + F])
                    et = e_pool.tile([128, F], FP32, tag="e")
                    nc.scalar.dma_start(et[:], e[p0 : p0 + 128, o, t0 : t0 + F])

                    eb = eb_pool.tile([128, F], BF16, tag="eb")
                    nc.scalar.copy(eb[:], et[:])
                    gt = g_pool.tile([128, F], BF16, tag="g")
                    nc.vector._custom_dve(
                        GMAX, out=gt[:], in0=kt[:], in1=et[:], **GMAX_C
                    )
                    ct = c_pool.tile([128, F], BF16, tag="c")
                    nc.vector.tensor_tensor(
                        ct[:], gt[:], acts[g][:, t0 : t0 + F], OP.mult
                    )
                    nc.scalar.activation(
                        Abig[:, t0 : t0 + F], ct[:], AF.Copy, bias=0.625, scale=1.0
                    )
                    nc.vector.tensor_tensor(
                        Bbig[:, t0 : t0 + F], ct[:], eb[:], OP.mult
                    )
                wbig = w_pool.tile([128, T], BF16, tag="w")
                pending.append((Abig, Bbig, wbig, o, g, ps))
                if len(pending) > 1:
                    flush_one()
            while len(pending) > 1:
                flush_one()
        while pending:
            flush_one()

    nc.compile()
    return nc


_NC_CACHE: list = []


def kernel(u_pre: np.ndarray, k_syn: np.ndarray, e_syn: np.ndarray) -> np.ndarray:
    if not _NC_CACHE:
        _NC_CACHE.append(_build_nc_v4())
    nc = _NC_CACHE[0]

    in_maps = []
    for i in range(N_CORES):
        lo, hi = i * NL, (i + 1) * NL
        in_maps.append(
            {
                "u": np.ascontiguousarray(u_pre[lo:hi, 0, :], dtype=np.float32),
                "k": np.ascontiguousarray(k_syn[lo:hi], dtype=np.float32),
                "e": np.ascontiguousarray(e_syn[lo:hi], dtype=np.float32),
            }
        )
    res = run_bass_kernel_spmd(nc, in_maps, list(range(N_CORES)))
    partials = np.stack([res.results[i]["out"] for i in range(N_CORES)])
    return partials.sum(axis=0, dtype=np.float32)


# revision 28
# speedup vs baseline: 1.1852x; 1.1852x over previous
"""Trainium2 Bass kernel for nn_NonSpikingOutput.

Reference semantics (N=4096 neurons, O=3 outputs, T=4096 steps):
    g_max = k/(e-k); act = clip(u, 0, 1); i_syn = g_max*act*(e - v)
    RK2 with i_syn frozen collapses to the per-element linear recurrence
        v_t = A_t * v_{t-1} + 0.075 * c_t * e_t,   A_t = 0.625 - 0.075*c_t
        c_t = act_t * k_t / (e_t - k_t)
    out[o, t] = sum_n v[n, o, t]

v4 design (measured HW rates per (128,2048) tile: TT bf16 1.22us, TT
f32/mixed 2.28us, TS bf16 0.68us, scalar_tensor_tensor/custom-DVE 2.28us
always, scan 4.39us any dtype, ACT 2.0us any dtype):

  DVE:  GMAX custom op  g~ = -0.075*k*recip_1nr(e-k)   [1 op, 2.28us]
        c~ = g~*act (bf16 TT), B~ = c~*eb (bf16 TT), scan(A, B~),
        act = clip(u) via dual TS (shared over the 3 outputs)
  ACT:  eb = bf16(e), A = c~ + 0.625 (Copy w/ bias), psum evac, carries
  PE:   colsum via (-1)-weights matmul -> psum = +0.075*sum_n v
Scans are software-pipelined one unit behind the TT chain so the ACT
A-compute latency never stalls the DVE.

Work scaled by -0.075: w~ = A*w~ + B~ accumulates -0.075*v, and the -1
matmul weights flip the sign back while summing over neurons.

Sharding: neuron dim N split across 8 cores (512 each); host sums the
per-core (O, T) partials.
"""

import sys
from contextlib import ExitStack

import numpy as np

sys.path.insert(0, "/opt/trn_rl_repo")

import concourse.bass as bass
import concourse.tile as tile
from concourse import bacc, mybir
from concourse.bass_utils import run_bass_kernel_spmd
from concourse import dve_ops as dops
from concourse.dve_spec import Spec, Bin, AluOp, Src0, Src1, C0, C1, C2, lower
from concourse.dve_spec import _has_src1 as _has_src1
from concourse.dve_uop import DveOpSpec

N_CORES = 8
N, O, T = 4096, 3, 4096
NL = N // N_CORES  # neurons per core
NG = NL // 128     # 128-partition neuron groups per core
F = 2048           # time-chunk (free dim) per tile
TC = T // F
FP32 = mybir.dt.float32
BF16 = mybir.dt.bfloat16
OP = mybir.AluOpType
AF = mybir.ActivationFunctionType


def _register_op(name: str, spec: Spec, subdim: bool = False):
    """Runtime-register a custom DVE op (name -> free opcode row)."""
    for o in dops.OPS:
        if o.name == name:
            return o
    row = dops._CUSTOM_DVE_ROW_BASE + len(dops.OPS)
    assert row < 0x20, "out of custom-DVE opcode rows"
    shas = {}
    for ver in ("v3", "v4"):
        try:
            uops = lower(spec, ver=ver)
            shas[ver] = DveOpSpec(
                name=name, opcode=row, uops=uops, rd1_en=_has_src1(spec)
            ).sha(ver)
        except Exception:
            pass
    assert "v3" in shas, "v3 lowering must succeed for TRN2"
    op = dops.DveOp(name, spec, subdim, shas)
    dops.OPS.append(op)
    dops.CUSTOM_DVE_SPECS[name] = spec
    dops._SUB_OPCODE_FOR_NAME[name] = row
    return op


# g~ = C2 * k * recip(e - k); recip = BITWISE_NOT exponent-flip seed + one
# Newton step (max rel err 1.7e-3 on d in (1,3); C0/C1 are the minimax pair).
_x = Src1 - Src0
_nx = Bin(AluOp.BITWISE_NOT, _x, _x)
_y0 = _nx * C0
_y1 = _y0 * (C1 - _x * _y0)
_GMAX_BODY = (Src0 * _y1) * C2


def _ref_gmax(in0, in1, c0, c1, c2):
    x = (in1 - in0).astype(np.float32)
    nx = (~x.view(np.int32)).view(np.float32)
    y0 = nx * np.float32(c0)
    y1 = y0 * (np.float32(c1) - x * y0)
    return (in0 * y1) * np.float32(c2)


GMAX = _register_op("GMAX_RECIP_ANT", Spec(body=_GMAX_BODY, reference=_ref_gmax))
GMAX_C = {"s0": -0.23549792, "s1": 2.0017324, "imm2": -0.075}


def _build_nc_v4() -> bass.Bass:
    nc = bacc.Bacc(
        "TRN2", target_bir_lowering=False, debug=False, num_devices=N_CORES
    )
    u = nc.dram_tensor("u", [NL, T], FP32, kind="ExternalInput")
    k = nc.dram_tensor("k", [NL, O, T], FP32, kind="ExternalInput")
    e = nc.dram_tensor("e", [NL, O, T], FP32, kind="ExternalInput")
    out = nc.dram_tensor("out", [O, T], FP32, kind="ExternalOutput")

    with tile.TileContext(nc) as tc, ExitStack() as ctx:
        const_pool = ctx.enter_context(tc.tile_pool(name="const", bufs=1))
        negones = const_pool.tile([128, 1], BF16)
        nc.vector.memset(negones[:], -1.0)
        # one fp32 carry column per (o, g) + one scratch col for strip chains
        carry = const_pool.tile([128, O * NG + 1], FP32)
        SCR = O * NG

        u_pool = ctx.enter_context(tc.tile_pool(name="u", bufs=1))
        act_pool = ctx.enter_context(tc.tile_pool(name="act", bufs=2 * NG))
        k_pool = ctx.enter_context(tc.tile_pool(name="k", bufs=3))
        e_pool = ctx.enter_context(tc.tile_pool(name="e", bufs=3))
        eb_pool = ctx.enter_context(tc.tile_pool(name="eb", bufs=3))
        g_pool = ctx.enter_context(tc.tile_pool(name="g", bufs=3))
        c_pool = ctx.enter_context(tc.tile_pool(name="c", bufs=3))
        a_pool = ctx.enter_context(tc.tile_pool(name="a", bufs=4))
        b_pool = ctx.enter_context(tc.tile_pool(name="b", bufs=4))
        w_pool = ctx.enter_context(tc.tile_pool(name="w", bufs=4))
        r_pool = ctx.enter_context(tc.tile_pool(name="r", bufs=1))
        ps_pool = ctx.enter_context(tc.tile_pool(name="ps", bufs=2, space="PSUM"))

        acts: dict[int, object] = {}
        acts_next: dict[int, object] = {}
        # software pipeline: scans run DEPTH units behind the TT chain so
        # every scan's inputs (incl. the ACT-produced A) are long-posted
        # by the time the scan issues - avoids per-scan semaphore latency.
        DEPTH = 2
        pending = []  # (A, B, w, ci, tci, o, g, ps)

        evacs = []  # deferred psum evacuations (deadline: ps reuse, far away)

        def drain_evac():
            pps, po, pt0 = evacs.pop(0)
            row = r_pool.tile([1, F], FP32, tag="row")
            nc.scalar.copy(row[:], pps[:])
            nc.sync.dma_start(out[po : po + 1, pt0 : pt0 + F], row[:, :])

        def flush_one():
            At, Bt, wt, ci, tci, o, g, ps = pending.pop(0)
            init = 0.0 if tci == 0 else carry[:, ci : ci + 1]
            nc.vector.tensor_tensor_scan(wt[:], At[:], Bt[:], init, OP.mult, OP.add)
            if tci < TC - 1:
                # carry on DVE ([128,1] ~0.1us): an ACT-side copy would wait
                # on the delayed scan INSIDE the ACT FIFO, head-of-line
                # blocking every later eb/A copy behind it
                nc.vector.tensor_scalar(
                    carry[:, ci : ci + 1], wt[:, F - 1 : F], 1.0, None, OP.mult
                )
            for s in range(F // 512):
                nc.tensor.matmul(
                    ps[0:1, s * 512 : (s + 1) * 512],
                    negones[:],
                    wt[:, s * 512 : (s + 1) * 512],
                    start=(g == 0),
                    stop=(g == NG - 1),
                )
            if g == NG - 1:
                evacs.append((ps, o, tci * F))
                if len(evacs) > 1:
                    drain_evac()

        for tci in range(TC):
            t0 = tci * F
            if tci > 0:
                acts = dict(acts_next)
                acts_next = {}
            for o in range(O):
                ps = ps_pool.tile([1, F], FP32, tag="ps", name=f"ps{tci}_{o}")
                for g in range(NG):
                    p0 = g * 128
                    kt = k_pool.tile([128, F], FP32, tag="k")
                    nc.sync.dma_start(kt[:], k[p0 : p0 + 128, o, t0 : t0 + F])
                    # e rides the Activation engine's HWDGE queue: splitting
                    # the 59MB input stream across two hardware queues keeps
                    # both k and e ahead of compute (one queue saturates)
                    et = e_pool.tile([128, F], FP32, tag="e")
                    nc.scalar.dma_start(et[:], e[p0 : p0 + 128, o, t0 : t0 + F])
                    if o == 0 and tci == 0:
                        ut = u_pool.tile([128, F], FP32, tag="u")
                        nc.sync.dma_start(ut[:], u[p0 : p0 + 128, t0 : t0 + F])

                    eb = eb_pool.tile([128, F], BF16, tag="eb")
                    nc.scalar.copy(eb[:], et[:])

                    gt = g_pool.tile([128, F], BF16, tag="g")
                    nc.vector._custom_dve(
                        GMAX, out=gt[:], in0=kt[:], in1=et[:], **GMAX_C
                    )
                    if o == 0 and tci == 0:
                        # u -> bf16 on ACT so the clip TS runs in 4x mode
                        ub = eb_pool.tile([128, F], BF16, tag="ub", name=f"ub{tci}_{g}")
                        nc.scalar.copy(ub[:], ut[:])
                        av = act_pool.tile([128, F], BF16, tag="act")
                        nc.vector.tensor_scalar(av[:], ub[:], 0.0, 1.0, OP.max, OP.min)
                        acts[g] = av
                    ct = c_pool.tile([128, F], BF16, tag="c")
                    nc.vector.tensor_tensor(ct[:], gt[:], acts[g][:], OP.mult)
                    # A = c~ + 0.625 on ACT (Copy w/ bias) to keep DVE lean
                    At = a_pool.tile([128, F], BF16, tag="a")
                    nc.scalar.activation(At[:], ct[:], AF.Copy, bias=0.625, scale=1.0)
                    Bt = b_pool.tile([128, F], BF16, tag="b")
                    nc.vector.tensor_tensor(Bt[:], ct[:], eb[:], OP.mult)

                    wt = w_pool.tile([128, F], BF16, tag="w")
                    ci = o * NG + g
                    pending.append((At, Bt, wt, ci, tci, o, g, ps))
                    # steady-state: flush DEPTH units behind; in the final
                    # group drain eagerly so trailing scans overlap the
                    # remaining TT work instead of serializing at the end.
                    last_group = tci == TC - 1 and o == O - 1
                    if len(pending) > (0 if last_group else DEPTH):
                        flush_one()
                    # prefetch next chunk's u + clip during this chunk's
                    # final output group: spreads the ACT/DVE o==0 burst
                    if o == O - 1 and tci < TC - 1:
                        nt0 = (tci + 1) * F
                        ut = u_pool.tile([128, F], FP32, tag="u")
                        nc.sync.dma_start(ut[:], u[p0 : p0 + 128, nt0 : nt0 + F])
                        ub = eb_pool.tile(
                            [128, F], BF16, tag="ub", name=f"ub{tci + 1}_{g}"
                        )
                        nc.scalar.copy(ub[:], ut[:])
                        av = act_pool.tile([128, F], BF16, tag="act")
                        nc.vector.tensor_scalar(av[:], ub[:], 0.0, 1.0, OP.max, OP.min)
                        acts_next[g] = av
        while pending:
            flush_one()
        while evacs:
            drain_evac()

    nc.compile()
    return nc


def _build_nc_v13() -> bass.Bass:
    """Full-T scans: one scan of (128, 4096) per (o, g) - no carry chain,
    half the scan launches, no chunk boundary. k/e/eb/g/c stay at F=2048
    (two halves per unit); A/B/w are double-width tiles whose halves are
    written independently, then consumed whole by the scan."""
    nc = bacc.Bacc(
        "TRN2", target_bir_lowering=False, debug=False, num_devices=N_CORES
    )
    u = nc.dram_tensor("u", [NL, T], FP32, kind="ExternalInput")
    k = nc.dram_tensor("k", [NL, O, T], FP32, kind="ExternalInput")
    e = nc.dram_tensor("e", [NL, O, T], FP32, kind="ExternalInput")
    out = nc.dram_tensor("out", [O, T], FP32, kind="ExternalOutput")

    with tile.TileContext(nc) as tc, ExitStack() as ctx:
        const_pool = ctx.enter_context(tc.tile_pool(name="const", bufs=1))
        negones = const_pool.tile([128, 1], BF16)
        nc.vector.memset(negones[:], -1.0)

        u_pool = ctx.enter_context(tc.tile_pool(name="u", bufs=1))
        ub_pool = ctx.enter_context(tc.tile_pool(name="ub", bufs=1))
        act_pool = ctx.enter_context(tc.tile_pool(name="act", bufs=NG))
        k_pool = ctx.enter_context(tc.tile_pool(name="k", bufs=3))
        e_pool = ctx.enter_context(tc.tile_pool(name="e", bufs=3))
        eb_pool = ctx.enter_context(tc.tile_pool(name="eb", bufs=3))
        g_pool = ctx.enter_context(tc.tile_pool(name="g", bufs=3))
        c_pool = ctx.enter_context(tc.tile_pool(name="c", bufs=3))
        a_pool = ctx.enter_context(tc.tile_pool(name="a", bufs=2))
        b_pool = ctx.enter_context(tc.tile_pool(name="b", bufs=2))
        w_pool = ctx.enter_context(tc.tile_pool(name="w", bufs=2))
        r_pool = ctx.enter_context(tc.tile_pool(name="r", bufs=1))
        ps_pool = ctx.enter_context(tc.tile_pool(name="ps", bufs=2, space="PSUM"))

        acts: dict[int, object] = {}
        pending = []  # (Abig, Bbig, wbig, g, ps)

        def flush_one():
            Abig, Bbig, wbig, oo, g, ps = pending.pop(0)
            nc.vector.tensor_tensor_scan(
                wbig[:], Abig[:], Bbig[:], 0.0, OP.mult, OP.add
            )
            for s in range(T // 512):
                nc.tensor.matmul(
                    ps[s // 4][0:1, (s % 4) * 512 : (s % 4) * 512 + 512],
                    negones[:],
                    wbig[:, s * 512 : (s + 1) * 512],
                    start=(g == 0),
                    stop=(g == NG - 1),
                )
            if g == NG - 1:
                for h in range(2):
                    row = r_pool.tile([1, F], FP32, tag="row")
                    nc.scalar.copy(row[:], ps[h][:])
                    nc.sync.dma_start(
                        out[oo : oo + 1, h * F : h * F + F], row[:, :]
                    )

        for o in range(O):
            ps = [
                ps_pool.tile([1, F], FP32, tag="ps", name=f"ps{o}_{h}")
                for h in range(2)
            ]
            for g in range(NG):
                p0 = g * 128
                if o == 0:
                    ut = u_pool.tile([128, T], FP32, tag="u")
                    nc.sync.dma_start(ut[:], u[p0 : p0 + 128, :])
                    ub = ub_pool.tile([128, T], BF16, tag="ub", name=f"ub{g}")
                    nc.scalar.copy(ub[:], ut[:])
                    av = act_pool.tile([128, T], BF16, tag="act")
                    nc.vector.tensor_scalar(av[:], ub[:], 0.0, 1.0, OP.max, OP.min)
                    acts[g] = av
                Abig = a_pool.tile([128, T], BF16, tag="a")
                Bbig = b_pool.tile([128, T], BF16, tag="b")
                for h in range(2):
                    t0 = h * F
                    kt = k_pool.tile([128, F], FP32, tag="k")
                    nc.sync.dma_start(kt[:], k[p0 : p0 + 128, o, t0 : t0 + F])
                    et = e_pool.tile([128, F], FP32, tag="e")
                    nc.scalar.dma_start(et[:], e[p0 : p0 + 128, o, t0 : t0 + F])

                    eb = eb_pool.tile([128, F], BF16, tag="eb")
                    nc.scalar.copy(eb[:], et[:])
                    gt = g_pool.tile([128, F], BF16, tag="g")
                    nc.vector._custom_dve(
                        GMAX, out=gt[:], in0=kt[:], in1=et[:], **GMAX_C
                    )
                    ct = c_pool.tile([128, F], BF16, tag="c")
                    nc.vector.tensor_tensor(
                        ct[:], gt[:], acts[g][:, t0 : t0 + F], OP.mult
                    )
                    nc.scalar.activation(
                        Abig[:, t0 : t0 + F], ct[:], AF.Copy, bias=0.625, scale=1.0
                    )
                    nc.vector.tensor_tensor(
                        Bbig[:, t0 : t0 + F], ct[:], eb[:], OP.mult
                    )
                wbig = w_pool.tile([128, T], BF16, tag="w")
                pending.append((Abig, Bbig, wbig, o, g, ps))
                if len(pending) > 1:
                    flush_one()
            while len(pending) > 1:
                flush_one()
        while pending:
            flush_one()

    nc.compile()
    return nc


_NC_CACHE: list = []


def kernel(u_pre: np.ndarray, k_syn: np.ndarray, e_syn: np.ndarray) -> np.ndarray:
    if not _NC_CACHE:
        _NC_CACHE.append(_build_nc_v4())
    nc = _NC_CACHE[0]

    in_maps = []
    for i in range(N_CORES):
        lo, hi = i * NL, (i + 1) * NL
        in_maps.append(
            {
                "u": np.ascontiguousarray(u_pre[lo:hi, 0, :], dtype=np.float32),
                "k": np.ascontiguousarray(k_syn[lo:hi], dtype=np.float32),
                "e": np.ascontiguousarray(e_syn[lo:hi], dtype=np.float32),
            }
        )
    res = run_bass_kernel_spmd(nc, in_maps, list(range(N_CORES)))
    partials = np.stack([res.results[i]["out"] for i in range(N_CORES)])
    return partials.sum(axis=0, dtype=np.float32)
